# revision 30
# baseline (speedup 1.0000x reference)
"""Trainium2 Bass kernel for a collision-grid social-LSTM model.

Math per frame t (N=512 agents, V=64 vehicles):
  social   = max_j grids_TTC[t, :, j, :]          # [N, 24]
  social_v = max_j grids_TTC_veh[t, :, j, :]      # [N, 24]
  e_in = relu(nodes @ W_in + b_in)                # nodes = input_data[:, [0,1,5..8]]
  e_t  = relu(social @ W_t + b_t)
  e_tv = relu(social_v @ W_tv + b_tv)
  gates = [e_in e_t e_tv] @ W_ih + b_ih + h @ W_hh + b_hh
  LSTM cell (i,f,g,o) -> h, c;  out = h @ W_out + b_out

Sharding: agent dim N split across 8 NeuronCores (64 rows each); weights
replicated; the T-scan stays sequential per core; no collectives.

Grid streaming: each frame slab [64i, 512j, 24s] is reshaped on host to
[128, 6144] fp16 with partition p = (j_half*64 + i) and free layout
s-major [24, 256j], so the DMA is one contiguous transfer and the
j-reduction runs as a halving tensor-max tree at the DVE's 2x fp16 rate.
A PE transpose + elementwise max merges the two j-halves and produces
social^T [24, 64] directly.

Everything downstream runs TRANSPOSED (feature dims on partitions, agent
rows on the free axis): gates^T chunks [128 gate-dims, 64 rows] accumulate
in PSUM from stationary weight-chunk matmuls; biases ride the sigmoid/tanh
activations as free per-partition bias vectors; h^T is written straight
into an SBUF history buffer (no per-step transposes or copies); matmul
operands are fp16 while PSUM accumulation and the LSTM cell state stay
fp32.
"""

import numpy as np

import concourse.tile as tile
from concourse import bacc, mybir

T, N, V = 19, 512, 64
F, E, R, O = 9, 128, 256, 5
S = 24
NCORES = 8
NL = N // NCORES          # 64 agent rows per core
ROWS = T * NL             # 1216 (t-major row index = t*NL + i)
PFREE = (N // 2) * S      # 6144 free elems per partition (ped)
VFREE = (V // 2) * S      # 768 (veh)

DT = mybir.dt.float32
GRID_DT = mybir.dt.float16   # dtype grids are staged in device DRAM
GRID_NP = np.float16
GEMM_DT = mybir.dt.float16   # matmul operand dtype (PSUM accumulates fp32)
GEMM_NP = np.float16

_NC_CACHE = {}


def build_nc(repeat=1, parts="all"):
    """Build + compile the per-core Bass module (identical on all cores).

    parts: "all" | "grids" (stream+reduce only) | "scan" (no grid streaming)
    — reduced variants are for cost-model experiments only.
    """
    key = (repeat, parts)
    if key in _NC_CACHE:
        return _NC_CACHE[key]

    nc = bacc.Bacc("TRN2", target_bir_lowering=False, debug=False,
                   num_devices=NCORES)
    dt = DT
    AF = mybir.ActivationFunctionType
    ALU = mybir.AluOpType
    AX = mybir.AxisListType

    # ---- DRAM I/O ----
    g_ped = nc.dram_tensor("g_ped", [T, 128, PFREE], GRID_DT, kind="ExternalInput")
    g_veh = nc.dram_tensor("g_veh", [T, 128, VFREE], GRID_DT, kind="ExternalInput")
    nodes_T = nc.dram_tensor("nodes_T", [6, ROWS], GEMM_DT, kind="ExternalInput")
    hT_init = nc.dram_tensor("hT_init", [R, NL], GEMM_DT, kind="ExternalInput")
    cT_init = nc.dram_tensor("cT_init", [R, NL], dt, kind="ExternalInput")
    w_in_d = nc.dram_tensor("w_in", [6, E], GEMM_DT, kind="ExternalInput")
    w_t_d = nc.dram_tensor("w_t", [S, E], GEMM_DT, kind="ExternalInput")
    w_tv_d = nc.dram_tensor("w_tv", [S, E], GEMM_DT, kind="ExternalInput")
    b_in_d = nc.dram_tensor("b_in_col", [E, 1], dt, kind="ExternalInput")
    b_t_d = nc.dram_tensor("b_t_col", [E, 1], dt, kind="ExternalInput")
    b_tv_d = nc.dram_tensor("b_tv_col", [E, 1], dt, kind="ExternalInput")
    w_ih_d = nc.dram_tensor("w_ih", [3 * E, 4 * R], GEMM_DT, kind="ExternalInput")
    w_hh_d = nc.dram_tensor("w_hh", [R, 4 * R], GEMM_DT, kind="ExternalInput")
    # b_ih + b_hh regrouped host-side as [128, 8] (chunk-major columns)
    b_sum_d = nc.dram_tensor("b_sum_cols", [E, 8], dt, kind="ExternalInput")
    w_out_d = nc.dram_tensor("w_out", [R, O], GEMM_DT, kind="ExternalInput")
    b_out_d = nc.dram_tensor("b_out_col", [O, 1], dt, kind="ExternalInput")
    ident_d = nc.dram_tensor("ident", [128, 128], dt, kind="ExternalInput")

    outs_d = nc.dram_tensor("outs_T", [O, ROWS], dt, kind="ExternalOutput")
    h_d = nc.dram_tensor("hT_out", [R, NL], GEMM_DT, kind="ExternalOutput")
    c_d = nc.dram_tensor("cT_out", [R, NL], dt, kind="ExternalOutput")

    with tile.TileContext(nc) as tc:
        import contextlib
        with contextlib.ExitStack() as ctx:
            consts = ctx.enter_context(tc.tile_pool(name="consts", bufs=1))
            grids = ctx.enter_context(tc.tile_pool(name="grids", bufs=4))
            gridsv = ctx.enter_context(tc.tile_pool(name="gridsv", bufs=4))
            tmp = ctx.enter_context(tc.tile_pool(name="tmp", bufs=3))
            lstm = ctx.enter_context(tc.tile_pool(name="lstm", bufs=2))
            ps_gates = ctx.enter_context(
                tc.tile_pool(name="ps_gates", bufs=2, space="PSUM"))
            ps_small = ctx.enter_context(
                tc.tile_pool(name="ps_small", bufs=4, space="PSUM"))

            # ---- load constants ----
            def cload(name, dram, shape, dtype=dt):
                t_ = consts.tile(shape, dtype, tag=name)
                nc.sync.dma_start(t_[:], dram[:])
                return t_

            w_in = cload("w_in", w_in_d, [6, E], GEMM_DT)
            w_t = cload("w_t", w_t_d, [S, E], GEMM_DT)
            w_tv = cload("w_tv", w_tv_d, [S, E], GEMM_DT)
            b_in = cload("b_in", b_in_d, [E, 1])
            b_t = cload("b_t", b_t_d, [E, 1])
            b_tv = cload("b_tv", b_tv_d, [E, 1])
            # W_ih [384, 1024] as three [128, 1024] k-slabs
            w_ih0 = consts.tile([E, 4 * R], GEMM_DT, tag="w_ih0")
            nc.sync.dma_start(w_ih0[:], w_ih_d[0:E, :])
            w_ih1 = consts.tile([E, 4 * R], GEMM_DT, tag="w_ih1")
            nc.sync.dma_start(w_ih1[:], w_ih_d[E:2 * E, :])
            w_ih2 = consts.tile([E, 4 * R], GEMM_DT, tag="w_ih2")
            nc.sync.dma_start(w_ih2[:], w_ih_d[2 * E:3 * E, :])
            w_hh0 = consts.tile([128, 4 * R], GEMM_DT, tag="w_hh0")
            nc.sync.dma_start(w_hh0[:], w_hh_d[0:128, :])
            w_hh1 = consts.tile([128, 4 * R], GEMM_DT, tag="w_hh1")
            nc.sync.dma_start(w_hh1[:], w_hh_d[128:256, :])
            b_sum = cload("b_sum", b_sum_d, [E, 8])
            w_out0 = consts.tile([128, O], GEMM_DT, tag="w_out0")
            nc.sync.dma_start(w_out0[:], w_out_d[0:128, :])
            w_out1 = consts.tile([128, O], GEMM_DT, tag="w_out1")
            nc.sync.dma_start(w_out1[:], w_out_d[128:256, :])
            b_out = cload("b_out", b_out_d, [O, 1])
            ident = cload("ident", ident_d, [128, 128])
            nodesT = cload("nodesT", nodes_T, [6, ROWS], GEMM_DT)

            # persistent activations (transposed, rows on free dim)
            eT_in = consts.tile([E, ROWS], GEMM_DT, tag="eT_in")
            eT_t = consts.tile([E, ROWS], GEMM_DT, tag="eT_t")
            eT_tv = consts.tile([E, ROWS], GEMM_DT, tag="eT_tv")
            socT = consts.tile([S, ROWS], GEMM_DT, tag="socT")
            socvT = consts.tile([S, ROWS], GEMM_DT, tag="socvT")
            outsT_sb = consts.tile([O, ROWS], dt, tag="outsT_sb")
            # h^T history: column block t holds h_{t-1}^T (block 0 = init)
            hist0 = consts.tile([128, (T + 1) * NL], GEMM_DT, tag="hist0")
            hist1 = consts.tile([128, (T + 1) * NL], GEMM_DT, tag="hist1")
            if parts.startswith("scan"):
                nc.vector.memset(socT[:], 0.5)
                nc.vector.memset(socvT[:], 0.5)

            for _rep in range(repeat):
                # ---- e_in for all rows upfront ----
                for n0 in range(0, ROWS, 512):
                    nn = min(512, ROWS - n0)
                    ps = ps_small.tile([128, 512], dt, tag="ps")
                    nc.tensor.matmul(ps[:, :nn], w_in[:], nodesT[:, n0:n0 + nn],
                                     start=True, stop=True)
                    nc.scalar.activation(eT_in[:, n0:n0 + nn], ps[:, :nn],
                                         AF.Relu, bias=b_in[:], scale=1.0)

                # ---- LSTM state init ----
                nc.sync.dma_start(hist0[:, 0:NL], hT_init[0:128, :])
                nc.sync.dma_start(hist1[:, 0:NL], hT_init[128:256, :])
                cT0 = lstm.tile([128, NL], dt, tag="cT0")
                nc.sync.dma_start(cT0[:], cT_init[0:128, :])
                cT1 = lstm.tile([128, NL], dt, tag="cT1")
                nc.sync.dma_start(cT1[:], cT_init[128:256, :])

                def emit_grids(t, cols):
                    # stream + reduce ped grid.  Free layout per partition
                    # is s-major [24, 256j]; halving tensor-max tree at the
                    # DVE's 2x fp16 rate, then one strided reduce finishes
                    # j=16 -> 1.
                    G = grids.tile([128, PFREE], GRID_DT, tag="G")
                    nc.sync.dma_start(G[:], g_ped[t])
                    scr = grids.tile([128, 5760], GRID_DT, tag="SCR")
                    lvls = [(G, 0, 256, scr, 0),          # -> [24,128] @0
                            (scr, 0, 128, scr, 3072),     # -> [24,64]  @3072
                            (scr, 3072, 64, scr, 4608),   # -> [24,32]  @4608
                            (scr, 4608, 32, scr, 5376)]   # -> [24,16]  @5376
                    for src_t, soff, jn, dst_t, doff in lvls:
                        jh = jn // 2
                        a = src_t[:, soff:soff + S * jn].rearrange(
                            "p (s j) -> p s j", s=S)
                        o = dst_t[:, doff:doff + S * jh].rearrange(
                            "p (s j) -> p s j", s=S)
                        nc.vector.tensor_max(o, a[:, :, 0:jh], a[:, :, jh:jn])
                    part = tmp.tile([128, S], dt, tag="part")
                    nc.vector.tensor_reduce(
                        part[:], scr[:, 5376:5760].rearrange(
                            "p (s j) -> p s j", s=S),
                        axis=AX.X, op=ALU.max)
                    psT = ps_small.tile([S, 128], dt, tag="ps")
                    nc.tensor.transpose(psT[:], part[:], ident[:])
                    sbT = tmp.tile([S, 128], dt, tag="sbT")
                    nc.scalar.copy(sbT[:], psT[:])
                    nc.vector.tensor_max(socT[:, cols], sbT[:, 0:NL],
                                         sbT[:, NL:2 * NL])
                    # stream + reduce veh grid (one tree level + reduce)
                    Gv = gridsv.tile([128, VFREE], GRID_DT, tag="Gv")
                    nc.sync.dma_start(Gv[:], g_veh[t])
                    scrv = gridsv.tile([128, VFREE // 2], GRID_DT, tag="SCRV")
                    av = Gv[:].rearrange("p (s j) -> p s j", s=S)
                    ov = scrv[:].rearrange("p (s j) -> p s j", s=S)
                    nc.vector.tensor_max(ov, av[:, :, 0:16], av[:, :, 16:32])
                    partv = tmp.tile([128, S], dt, tag="partv")
                    nc.vector.tensor_reduce(
                        partv[:], scrv[:].rearrange("p (s j) -> p s j", s=S),
                        axis=AX.X, op=ALU.max)
                    psTv = ps_small.tile([S, 128], dt, tag="ps")
                    nc.tensor.transpose(psTv[:], partv[:], ident[:])
                    sbTv = tmp.tile([S, 128], dt, tag="sbTv")
                    nc.scalar.copy(sbTv[:], psTv[:])
                    nc.vector.tensor_max(socvT[:, cols], sbTv[:, 0:NL],
                                         sbTv[:, NL:2 * NL])

                def gcols(t):
                    return slice(t * NL, (t + 1) * NL)

                LA = 2   # grid pipeline lookahead (frames) vs the scan
                if parts == "all":
                    for t in range(min(LA, T)):
                        emit_grids(t, gcols(t))

                for t in range(T):
                    cols = slice(t * NL, (t + 1) * NL)
                    hcols = slice(t * NL, (t + 1) * NL)         # h_{t-1}
                    hncols = slice((t + 1) * NL, (t + 2) * NL)  # h_t

                    if parts == "grids":
                        emit_grids(t, cols)
                        continue
                    if parts == "all" and t + LA < T:
                        emit_grids(t + LA, gcols(t + LA))

                    # ---- social embeddings (transposed) ----
                    pse = ps_small.tile([E, NL], dt, tag="ps")
                    nc.tensor.matmul(pse[:], w_t[:], socT[:, cols],
                                     start=True, stop=True)
                    nc.scalar.activation(eT_t[:, cols], pse[:], AF.Relu,
                                         bias=b_t[:], scale=1.0)
                    psev = ps_small.tile([E, NL], dt, tag="ps")
                    nc.tensor.matmul(psev[:], w_tv[:], socvT[:, cols],
                                     start=True, stop=True)
                    nc.scalar.activation(eT_tv[:, cols], psev[:], AF.Relu,
                                         bias=b_tv[:], scale=1.0)

                    # ---- gates^T in 8 chunks of 128 gate-dims.
                    # chunk c covers gate cols [128c, 128c+128);
                    # i = chunks 0,1; f = 2,3; g = 4,5; o = 6,7.
                    # Two PSUM tiles hold 4 chunks each (side by side);
                    # each chunk's accumulation group is contiguous. ----
                    gA = ps_gates.tile([128, 4 * NL], dt, tag="gA")
                    gB = ps_gates.tile([128, 4 * NL], dt, tag="gB")
                    act = tmp.tile([128, 8 * NL], dt, tag="act")
                    for c in range(8):
                        g_ps = gA if c < 4 else gB
                        out = g_ps[:, (c % 4) * NL:(c % 4) * NL + NL]
                        wsl = slice(c * 128, (c + 1) * 128)
                        nc.tensor.matmul(out, w_ih0[:, wsl], eT_in[:, cols],
                                         start=True, stop=False)
                        nc.tensor.matmul(out, w_ih1[:, wsl], eT_t[:, cols],
                                         start=False, stop=False)
                        nc.tensor.matmul(out, w_ih2[:, wsl], eT_tv[:, cols],
                                         start=False, stop=False)
                        nc.tensor.matmul(out, w_hh0[:, wsl], hist0[:, hcols],
                                         start=False, stop=False)
                        nc.tensor.matmul(out, w_hh1[:, wsl], hist1[:, hcols],
                                         start=False, stop=True)
                        # sigmoid for i,f,o; tanh for g; bias rides along
                        func = AF.Tanh if c in (4, 5) else AF.Sigmoid
                        nc.scalar.activation(act[:, c * NL:(c + 1) * NL], out,
                                             func, bias=b_sum[:, c:c + 1],
                                             scale=1.0)

                    # act cols: i0 i1 f0 f1 g0 g1 o0 o1 (64 each)
                    def acol(c):
                        return act[:, c * NL:(c + 1) * NL]

                    # ---- c_new^T = sig_f*c + sig_i*tan_g (per r-half) ----
                    cT0_n = lstm.tile([128, NL], dt, tag="cT0")
                    cT1_n = lstm.tile([128, NL], dt, tag="cT1")
                    tanc = tmp.tile([128, 2 * NL], dt, tag="tanc")
                    for half, (c_old, c_new) in enumerate(
                            ((cT0, cT0_n), (cT1, cT1_n))):
                        t1_ = tmp.tile([128, NL], dt, tag=f"t1_{half}")
                        nc.vector.tensor_mul(t1_[:], acol(2 + half), c_old[:])
                        t2_ = tmp.tile([128, NL], dt, tag=f"t2_{half}")
                        nc.vector.tensor_mul(t2_[:], acol(0 + half),
                                             acol(4 + half))
                        nc.vector.tensor_add(c_new[:], t1_[:], t2_[:])
                        nc.scalar.activation(
                            tanc[:, half * NL:(half + 1) * NL], c_new[:],
                            AF.Tanh)
                        # h^T half -> history (fp16 for the next matmul)
                        hist = hist0 if half == 0 else hist1
                        nc.vector.tensor_mul(hist[:, hncols], acol(6 + half),
                                             tanc[:, half * NL:(half + 1) * NL])
                    cT0, cT1 = cT0_n, cT1_n

                    # ---- out_t^T = W_out^T h_t + b_out ----
                    pso = ps_small.tile([O, NL], dt, tag="ps")
                    nc.tensor.matmul(pso[:], w_out0[:], hist0[:, hncols],
                                     start=True, stop=False)
                    nc.tensor.matmul(pso[:], w_out1[:], hist1[:, hncols],
                                     start=False, stop=True)
                    nc.scalar.activation(outsT_sb[:, cols], pso[:],
                                         AF.Identity, bias=b_out[:], scale=1.0)

            # ---- writeback ----
            if parts == "grids":
                nc.sync.dma_start(outs_d[0:O, 0:NL], socT[0:O, 0:NL])
            else:
                nc.sync.dma_start(outs_d[:], outsT_sb[:])
                nc.sync.dma_start(h_d[0:128, :], hist0[:, T * NL:(T + 1) * NL])
                nc.sync.dma_start(h_d[128:256, :], hist1[:, T * NL:(T + 1) * NL])
                nc.sync.dma_start(c_d[0:128, :], cT0[:])
                nc.sync.dma_start(c_d[128:256, :], cT1[:])

    nc.compile()
    _NC_CACHE[key] = nc
    return nc


def shard_inputs(inputs):
    """Full inputs -> list of 8 per-core input maps (numpy, C-contiguous)."""
    f32 = np.float32
    inp = np.asarray(inputs["input_data"], f32)
    gttc = np.asarray(inputs["grids_TTC"], f32)
    gttcv = np.asarray(inputs["grids_TTC_veh"], f32)
    h0 = np.asarray(inputs["hidden_states"], f32)
    c0 = np.asarray(inputs["cell_states"], f32)

    w_in = np.ascontiguousarray(np.asarray(inputs["W_in"], f32).astype(GEMM_NP))
    w_t = np.ascontiguousarray(np.asarray(inputs["W_t"], f32).astype(GEMM_NP))
    w_tv = np.ascontiguousarray(np.asarray(inputs["W_tv"], f32).astype(GEMM_NP))
    w_ih = np.ascontiguousarray(np.asarray(inputs["W_ih"], f32).astype(GEMM_NP))
    w_hh = np.ascontiguousarray(np.asarray(inputs["W_hh"], f32).astype(GEMM_NP))
    w_out = np.ascontiguousarray(np.asarray(inputs["W_out"], f32).astype(GEMM_NP))
    b_in = np.ascontiguousarray(np.asarray(inputs["b_in"], f32).reshape(E, 1))
    b_t = np.ascontiguousarray(np.asarray(inputs["b_t"], f32).reshape(E, 1))
    b_tv = np.ascontiguousarray(np.asarray(inputs["b_tv"], f32).reshape(E, 1))
    # per-chunk bias columns: [128, 8], chunk c = gate cols [128c, 128c+128)
    b_sum = np.ascontiguousarray(
        (np.asarray(inputs["b_ih"], f32) + np.asarray(inputs["b_hh"], f32))
        .reshape(8, 128).T)
    b_out = np.ascontiguousarray(np.asarray(inputs["b_out"], f32).reshape(O, 1))
    ident = np.eye(128, dtype=f32)

    shared = dict(w_in=w_in, w_t=w_t, w_tv=w_tv, w_ih=w_ih, w_hh=w_hh,
                  w_out=w_out, b_in_col=b_in, b_t_col=b_t, b_tv_col=b_tv,
                  b_sum_cols=b_sum, b_out_col=b_out, ident=ident)

    in_maps = []
    for d in range(NCORES):
        i0 = d * NL
        gp = gttc[:, i0:i0 + NL]                       # [T, 64, 512, 24]
        gp = np.ascontiguousarray(
            gp.reshape(T, NL, 2, N // 2, S).transpose(0, 2, 1, 4, 3)
            .reshape(T, 128, PFREE).astype(GRID_NP))
        gv = gttcv[:, i0:i0 + NL]                      # [T, 64, 64, 24]
        gv = np.ascontiguousarray(
            gv.reshape(T, NL, 2, V // 2, S).transpose(0, 2, 1, 4, 3)
            .reshape(T, 128, VFREE).astype(GRID_NP))
        nd = inp[:, i0:i0 + NL][:, :, [0, 1, 5, 6, 7, 8]]  # [T, 64, 6]
        nodes_T = np.ascontiguousarray(nd.reshape(ROWS, 6).T.astype(GEMM_NP))
        in_maps.append(dict(
            g_ped=gp, g_veh=gv, nodes_T=nodes_T,
            hT_init=np.ascontiguousarray(h0[i0:i0 + NL].T.astype(GEMM_NP)),
            cT_init=np.ascontiguousarray(c0[i0:i0 + NL].T),
            **shared))
    return in_maps


def gather_outputs(results):
    outs, hs, cs = [], [], []
    for r in results:
        # outs_T [5, 19*64] -> [19, 64, 5]
        o = r["outs_T"].reshape(O, T, NL).transpose(1, 2, 0)
        outs.append(o)
        hs.append(np.ascontiguousarray(r["hT_out"].T.astype(np.float32)))
        cs.append(np.ascontiguousarray(r["cT_out"].T))
    return (np.concatenate(outs, axis=1),
            np.concatenate(hs, axis=0),
            np.concatenate(cs, axis=0))


def kernel(**inputs):
    from concourse.bass_utils import run_bass_kernel_spmd
    nc = build_nc()
    in_maps = shard_inputs(inputs)
    res = run_bass_kernel_spmd(nc, in_maps, core_ids=list(range(NCORES)))
    return gather_outputs(res.results)


# revision 32
# speedup vs baseline: 1.0390x; 1.0390x over previous
"""Trainium2 Bass kernel for a collision-grid social-LSTM model.

Math per frame t (N=512 agents, V=64 vehicles):
  social   = max_j grids_TTC[t, :, j, :]          # [N, 24]
  social_v = max_j grids_TTC_veh[t, :, j, :]      # [N, 24]
  e_in = relu(nodes @ W_in + b_in)                # nodes = input_data[:, [0,1,5..8]]
  e_t  = relu(social @ W_t + b_t)
  e_tv = relu(social_v @ W_tv + b_tv)
  gates = [e_in e_t e_tv] @ W_ih + b_ih + h @ W_hh + b_hh
  LSTM cell (i,f,g,o) -> h, c;  out = h @ W_out + b_out

Sharding: agent dim N split across 8 NeuronCores (64 rows each); weights
replicated; the T-scan stays sequential per core; no collectives.

Grid streaming: each frame slab [64i, 512j, 24s] is reshaped on host to
[128, 6144] fp16 with partition p = (j_half*64 + i) and free layout
s-major [24, 256j], so the DMA is one contiguous transfer and the
j-reduction runs as a halving tensor-max tree at the DVE's 2x fp16 rate.
A PE transpose + elementwise max merges the two j-halves and produces
social^T [24, 64] directly.

Everything downstream runs TRANSPOSED (feature dims on partitions, agent
rows on the free axis): gates^T chunks [128 gate-dims, 64 rows] accumulate
in PSUM from stationary weight-chunk matmuls; biases ride the sigmoid/tanh
activations as free per-partition bias vectors; h^T is written straight
into an SBUF history buffer (no per-step transposes or copies); matmul
operands are fp16 while PSUM accumulation and the LSTM cell state stay
fp32.
"""

import numpy as np

import concourse.tile as tile
from concourse import bacc, mybir

T, N, V = 19, 512, 64
F, E, R, O = 9, 128, 256, 5
S = 24
NCORES = 8
NL = N // NCORES          # 64 agent rows per core
ROWS = T * NL             # 1216 (t-major row index = t*NL + i)
PFREE = (N // 2) * S      # 6144 free elems per partition (ped)
VFREE = (V // 2) * S      # 768 (veh)

DT = mybir.dt.float32
GRID_DT = mybir.dt.float16   # dtype grids are staged in device DRAM
GRID_NP = np.float16
GEMM_DT = mybir.dt.float16   # matmul operand dtype (PSUM accumulates fp32)
GEMM_NP = np.float16

_NC_CACHE = {}


def build_nc(repeat=1, parts="all"):
    """Build + compile the per-core Bass module (identical on all cores).

    parts: "all" | "grids" (stream+reduce only) | "scan" (no grid streaming)
    — reduced variants are for cost-model experiments only.
    """
    key = (repeat, parts)
    if key in _NC_CACHE:
        return _NC_CACHE[key]

    nc = bacc.Bacc("TRN2", target_bir_lowering=False, debug=False,
                   num_devices=NCORES)
    dt = DT
    AF = mybir.ActivationFunctionType
    ALU = mybir.AluOpType
    AX = mybir.AxisListType

    # ---- DRAM I/O ----
    g_ped = nc.dram_tensor("g_ped", [T, 128, PFREE], GRID_DT, kind="ExternalInput")
    g_veh = nc.dram_tensor("g_veh", [T, 128, VFREE], GRID_DT, kind="ExternalInput")
    nodes_T = nc.dram_tensor("nodes_T", [6, ROWS], GEMM_DT, kind="ExternalInput")
    hT_init = nc.dram_tensor("hT_init", [R, NL], GEMM_DT, kind="ExternalInput")
    cT_init = nc.dram_tensor("cT_init", [R, NL], dt, kind="ExternalInput")
    w_in_d = nc.dram_tensor("w_in", [6, E], GEMM_DT, kind="ExternalInput")
    w_t_d = nc.dram_tensor("w_t", [S, E], GEMM_DT, kind="ExternalInput")
    w_tv_d = nc.dram_tensor("w_tv", [S, E], GEMM_DT, kind="ExternalInput")
    b_in_d = nc.dram_tensor("b_in_col", [E, 1], dt, kind="ExternalInput")
    b_t_d = nc.dram_tensor("b_t_col", [E, 1], dt, kind="ExternalInput")
    b_tv_d = nc.dram_tensor("b_tv_col", [E, 1], dt, kind="ExternalInput")
    w_ih_d = nc.dram_tensor("w_ih", [3 * E, 4 * R], GEMM_DT, kind="ExternalInput")
    w_hh_d = nc.dram_tensor("w_hh", [R, 4 * R], GEMM_DT, kind="ExternalInput")
    # b_ih + b_hh regrouped host-side as [128, 8] (chunk-major columns)
    b_sum_d = nc.dram_tensor("b_sum_cols", [E, 8], dt, kind="ExternalInput")
    w_out_d = nc.dram_tensor("w_out", [R, O], GEMM_DT, kind="ExternalInput")
    b_out_d = nc.dram_tensor("b_out_col", [O, 1], dt, kind="ExternalInput")
    ident_d = nc.dram_tensor("ident", [128, 128], dt, kind="ExternalInput")

    outs_d = nc.dram_tensor("outs_T", [O, ROWS], dt, kind="ExternalOutput")
    h_d = nc.dram_tensor("hT_out", [R, NL], GEMM_DT, kind="ExternalOutput")
    c_d = nc.dram_tensor("cT_out", [R, NL], dt, kind="ExternalOutput")

    with tile.TileContext(nc) as tc:
        import contextlib
        with contextlib.ExitStack() as ctx:
            consts = ctx.enter_context(tc.tile_pool(name="consts", bufs=1))
            grids = ctx.enter_context(tc.tile_pool(name="grids", bufs=4))
            gridsv = ctx.enter_context(tc.tile_pool(name="gridsv", bufs=4))
            tmp = ctx.enter_context(tc.tile_pool(name="tmp", bufs=3))
            lstm = ctx.enter_context(tc.tile_pool(name="lstm", bufs=2))
            ps_gates = ctx.enter_context(
                tc.tile_pool(name="ps_gates", bufs=2, space="PSUM"))
            ps_small = ctx.enter_context(
                tc.tile_pool(name="ps_small", bufs=4, space="PSUM"))

            # ---- load constants ----
            def cload(name, dram, shape, dtype=dt):
                t_ = consts.tile(shape, dtype, tag=name)
                nc.sync.dma_start(t_[:], dram[:])
                return t_

            w_in = cload("w_in", w_in_d, [6, E], GEMM_DT)
            w_t = cload("w_t", w_t_d, [S, E], GEMM_DT)
            w_tv = cload("w_tv", w_tv_d, [S, E], GEMM_DT)
            b_in = cload("b_in", b_in_d, [E, 1])
            b_t = cload("b_t", b_t_d, [E, 1])
            b_tv = cload("b_tv", b_tv_d, [E, 1])
            # W_ih [384, 1024] as three [128, 1024] k-slabs
            w_ih0 = consts.tile([E, 4 * R], GEMM_DT, tag="w_ih0")
            nc.sync.dma_start(w_ih0[:], w_ih_d[0:E, :])
            w_ih1 = consts.tile([E, 4 * R], GEMM_DT, tag="w_ih1")
            nc.sync.dma_start(w_ih1[:], w_ih_d[E:2 * E, :])
            w_ih2 = consts.tile([E, 4 * R], GEMM_DT, tag="w_ih2")
            nc.sync.dma_start(w_ih2[:], w_ih_d[2 * E:3 * E, :])
            w_hh0 = consts.tile([128, 4 * R], GEMM_DT, tag="w_hh0")
            nc.sync.dma_start(w_hh0[:], w_hh_d[0:128, :])
            w_hh1 = consts.tile([128, 4 * R], GEMM_DT, tag="w_hh1")
            nc.sync.dma_start(w_hh1[:], w_hh_d[128:256, :])
            b_sum = cload("b_sum", b_sum_d, [E, 8])
            w_out0 = consts.tile([128, O], GEMM_DT, tag="w_out0")
            nc.sync.dma_start(w_out0[:], w_out_d[0:128, :])
            w_out1 = consts.tile([128, O], GEMM_DT, tag="w_out1")
            nc.sync.dma_start(w_out1[:], w_out_d[128:256, :])
            b_out = cload("b_out", b_out_d, [O, 1])
            ident = cload("ident", ident_d, [128, 128])
            nodesT = cload("nodesT", nodes_T, [6, ROWS], GEMM_DT)

            # persistent activations (transposed, rows on free dim)
            eT_in = consts.tile([E, ROWS], GEMM_DT, tag="eT_in")
            eT_t = consts.tile([E, ROWS], GEMM_DT, tag="eT_t")
            eT_tv = consts.tile([E, ROWS], GEMM_DT, tag="eT_tv")
            socT = consts.tile([S, ROWS], GEMM_DT, tag="socT")
            socvT = consts.tile([S, ROWS], GEMM_DT, tag="socvT")
            outsT_sb = consts.tile([O, ROWS], dt, tag="outsT_sb")
            # h^T history: column block t holds h_{t-1}^T (block 0 = init)
            hist0 = consts.tile([128, (T + 1) * NL], GEMM_DT, tag="hist0")
            hist1 = consts.tile([128, (T + 1) * NL], GEMM_DT, tag="hist1")
            if parts.startswith("scan"):
                nc.vector.memset(socT[:], 0.5)
                nc.vector.memset(socvT[:], 0.5)

            for _rep in range(repeat):
                # ---- e_in for all rows upfront ----
                for n0 in range(0, ROWS, 512):
                    nn = min(512, ROWS - n0)
                    ps = ps_small.tile([128, 512], dt, tag="ps")
                    nc.tensor.matmul(ps[:, :nn], w_in[:], nodesT[:, n0:n0 + nn],
                                     start=True, stop=True)
                    nc.scalar.activation(eT_in[:, n0:n0 + nn], ps[:, :nn],
                                         AF.Relu, bias=b_in[:], scale=1.0)

                # ---- LSTM state init ----
                nc.sync.dma_start(hist0[:, 0:NL], hT_init[0:128, :])
                nc.sync.dma_start(hist1[:, 0:NL], hT_init[128:256, :])
                cT0 = lstm.tile([128, NL], dt, tag="cT0")
                nc.sync.dma_start(cT0[:], cT_init[0:128, :])
                cT1 = lstm.tile([128, NL], dt, tag="cT1")
                nc.sync.dma_start(cT1[:], cT_init[128:256, :])

                def emit_grids(t, cols):
                    # stream + reduce ped grid.  Free layout per partition
                    # is s-major [24, 256j]; halving tensor-max tree at the
                    # DVE's 2x fp16 rate, then one strided reduce finishes
                    # j=16 -> 1.
                    G = grids.tile([128, PFREE], GRID_DT, tag="G")
                    nc.sync.dma_start(G[:], g_ped[t])
                    scr = grids.tile([128, 5760], GRID_DT, tag="SCR")
                    lvls = [(G, 0, 256, scr, 0),          # -> [24,128] @0
                            (scr, 0, 128, scr, 3072),     # -> [24,64]  @3072
                            (scr, 3072, 64, scr, 4608),   # -> [24,32]  @4608
                            (scr, 4608, 32, scr, 5376)]   # -> [24,16]  @5376
                    for src_t, soff, jn, dst_t, doff in lvls:
                        jh = jn // 2
                        a = src_t[:, soff:soff + S * jn].rearrange(
                            "p (s j) -> p s j", s=S)
                        o = dst_t[:, doff:doff + S * jh].rearrange(
                            "p (s j) -> p s j", s=S)
                        nc.vector.tensor_max(o, a[:, :, 0:jh], a[:, :, jh:jn])
                    part = tmp.tile([128, S], dt, tag="part")
                    nc.vector.tensor_reduce(
                        part[:], scr[:, 5376:5760].rearrange(
                            "p (s j) -> p s j", s=S),
                        axis=AX.X, op=ALU.max)
                    psT = ps_small.tile([S, 128], dt, tag="ps")
                    nc.tensor.transpose(psT[:], part[:], ident[:])
                    sbT = tmp.tile([S, 128], dt, tag="sbT")
                    nc.scalar.copy(sbT[:], psT[:])
                    nc.vector.tensor_max(socT[:, cols], sbT[:, 0:NL],
                                         sbT[:, NL:2 * NL])
                    # stream + reduce veh grid (one tree level + reduce)
                    Gv = gridsv.tile([128, VFREE], GRID_DT, tag="Gv")
                    nc.sync.dma_start(Gv[:], g_veh[t])
                    scrv = gridsv.tile([128, VFREE // 2], GRID_DT, tag="SCRV")
                    av = Gv[:].rearrange("p (s j) -> p s j", s=S)
                    ov = scrv[:].rearrange("p (s j) -> p s j", s=S)
                    nc.vector.tensor_max(ov, av[:, :, 0:16], av[:, :, 16:32])
                    partv = tmp.tile([128, S], dt, tag="partv")
                    nc.vector.tensor_reduce(
                        partv[:], scrv[:].rearrange("p (s j) -> p s j", s=S),
                        axis=AX.X, op=ALU.max)
                    psTv = ps_small.tile([S, 128], dt, tag="ps")
                    nc.tensor.transpose(psTv[:], partv[:], ident[:])
                    sbTv = tmp.tile([S, 128], dt, tag="sbTv")
                    nc.scalar.copy(sbTv[:], psTv[:])
                    nc.vector.tensor_max(socvT[:, cols], sbTv[:, 0:NL],
                                         sbTv[:, NL:2 * NL])
                    # social embeddings (transposed)
                    pse = ps_small.tile([E, NL], dt, tag="ps")
                    nc.tensor.matmul(pse[:], w_t[:], socT[:, cols],
                                     start=True, stop=True)
                    nc.scalar.activation(eT_t[:, cols], pse[:], AF.Relu,
                                         bias=b_t[:], scale=1.0)
                    psev = ps_small.tile([E, NL], dt, tag="ps")
                    nc.tensor.matmul(psev[:], w_tv[:], socvT[:, cols],
                                     start=True, stop=True)
                    nc.scalar.activation(eT_tv[:, cols], psev[:], AF.Relu,
                                         bias=b_tv[:], scale=1.0)

                def emit_xw(t):
                    # input-side GEMM for step t: one accumulation group per
                    # PSUM bank (single start on the bank's first matmul; the
                    # h-side matmuls of step t will extend and stop it).
                    # Emitted during step t-1's ACT/DVE chain so the PE
                    # never idles at the h dependency.
                    cols = slice(t * NL, (t + 1) * NL)
                    gA = ps_gates.tile([128, 4 * NL], dt, tag="gA")
                    gB = ps_gates.tile([128, 4 * NL], dt, tag="gB")
                    for c in range(8):
                        g_ps = gA if c < 4 else gB
                        out = g_ps[:, (c % 4) * NL:(c % 4) * NL + NL]
                        wsl = slice(c * 128, (c + 1) * 128)
                        nc.tensor.matmul(out, w_ih0[:, wsl], eT_in[:, cols],
                                         start=(c % 4 == 0), stop=False)
                        nc.tensor.matmul(out, w_ih1[:, wsl], eT_t[:, cols],
                                         start=False, stop=False)
                        nc.tensor.matmul(out, w_ih2[:, wsl], eT_tv[:, cols],
                                         start=False, stop=False)
                    return gA, gB

                def gcols(t):
                    return slice(t * NL, (t + 1) * NL)

                LA = 2   # grid pipeline lookahead (frames) vs the scan
                if parts == "all":
                    for t in range(min(LA, T)):
                        emit_grids(t, gcols(t))
                    gA, gB = emit_xw(0)

                for t in range(T):
                    cols = slice(t * NL, (t + 1) * NL)
                    hcols = slice(t * NL, (t + 1) * NL)         # h_{t-1}
                    hncols = slice((t + 1) * NL, (t + 2) * NL)  # h_t

                    if parts == "grids":
                        emit_grids(t, cols)
                        continue
                    if parts == "all" and t + LA < T:
                        emit_grids(t + LA, gcols(t + LA))

                    # ---- h-side GEMM extends the per-bank groups opened
                    # by emit_xw(t); stop on each bank's last matmul ----
                    act = tmp.tile([128, 8 * NL], dt, tag="act")
                    for c in range(8):
                        g_ps = gA if c < 4 else gB
                        out = g_ps[:, (c % 4) * NL:(c % 4) * NL + NL]
                        wsl = slice(c * 128, (c + 1) * 128)
                        nc.tensor.matmul(out, w_hh0[:, wsl], hist0[:, hcols],
                                         start=False, stop=False)
                        nc.tensor.matmul(out, w_hh1[:, wsl], hist1[:, hcols],
                                         start=False, stop=(c % 4 == 3))
                        if c % 4 == 3:
                            # bank group closed: activations for its 4 chunks
                            for cc in range(c - 3, c + 1):
                                o2 = g_ps[:, (cc % 4) * NL:(cc % 4) * NL + NL]
                                func = (AF.Tanh if cc in (4, 5)
                                        else AF.Sigmoid)
                                nc.scalar.activation(
                                    act[:, cc * NL:(cc + 1) * NL], o2,
                                    func, bias=b_sum[:, cc:cc + 1], scale=1.0)

                    # act cols: i0 i1 f0 f1 g0 g1 o0 o1 (64 each)
                    def acol(c):
                        return act[:, c * NL:(c + 1) * NL]

                    # ---- c_new^T = sig_f*c + sig_i*tan_g (per r-half) ----
                    cT0_n = lstm.tile([128, NL], dt, tag="cT0")
                    cT1_n = lstm.tile([128, NL], dt, tag="cT1")
                    tanc = tmp.tile([128, 2 * NL], dt, tag="tanc")
                    for half, (c_old, c_new) in enumerate(
                            ((cT0, cT0_n), (cT1, cT1_n))):
                        t1_ = tmp.tile([128, NL], dt, tag=f"t1_{half}")
                        nc.vector.tensor_mul(t1_[:], acol(2 + half), c_old[:])
                        t2_ = tmp.tile([128, NL], dt, tag=f"t2_{half}")
                        nc.vector.tensor_mul(t2_[:], acol(0 + half),
                                             acol(4 + half))
                        nc.vector.tensor_add(c_new[:], t1_[:], t2_[:])
                        nc.scalar.activation(
                            tanc[:, half * NL:(half + 1) * NL], c_new[:],
                            AF.Tanh)
                        # h^T half -> history (fp16 for the next matmul)
                        hist = hist0 if half == 0 else hist1
                        nc.vector.tensor_mul(hist[:, hncols], acol(6 + half),
                                             tanc[:, half * NL:(half + 1) * NL])
                    cT0, cT1 = cT0_n, cT1_n

                    # ---- out_t^T = W_out^T h_t + b_out ----
                    pso = ps_small.tile([O, NL], dt, tag="ps")
                    nc.tensor.matmul(pso[:], w_out0[:], hist0[:, hncols],
                                     start=True, stop=False)
                    nc.tensor.matmul(pso[:], w_out1[:], hist1[:, hncols],
                                     start=False, stop=True)
                    nc.scalar.activation(outsT_sb[:, cols], pso[:],
                                         AF.Identity, bias=b_out[:], scale=1.0)

                    if t + 1 < T:
                        gA, gB = emit_xw(t + 1)

            # ---- writeback ----
            if parts == "grids":
                nc.sync.dma_start(outs_d[0:O, 0:NL], socT[0:O, 0:NL])
            else:
                nc.sync.dma_start(outs_d[:], outsT_sb[:])
                nc.sync.dma_start(h_d[0:128, :], hist0[:, T * NL:(T + 1) * NL])
                nc.sync.dma_start(h_d[128:256, :], hist1[:, T * NL:(T + 1) * NL])
                nc.sync.dma_start(c_d[0:128, :], cT0[:])
                nc.sync.dma_start(c_d[128:256, :], cT1[:])

    nc.compile()
    _NC_CACHE[key] = nc
    return nc


def shard_inputs(inputs):
    """Full inputs -> list of 8 per-core input maps (numpy, C-contiguous)."""
    f32 = np.float32
    inp = np.asarray(inputs["input_data"], f32)
    gttc = np.asarray(inputs["grids_TTC"], f32)
    gttcv = np.asarray(inputs["grids_TTC_veh"], f32)
    h0 = np.asarray(inputs["hidden_states"], f32)
    c0 = np.asarray(inputs["cell_states"], f32)

    w_in = np.ascontiguousarray(np.asarray(inputs["W_in"], f32).astype(GEMM_NP))
    w_t = np.ascontiguousarray(np.asarray(inputs["W_t"], f32).astype(GEMM_NP))
    w_tv = np.ascontiguousarray(np.asarray(inputs["W_tv"], f32).astype(GEMM_NP))
    w_ih = np.ascontiguousarray(np.asarray(inputs["W_ih"], f32).astype(GEMM_NP))
    w_hh = np.ascontiguousarray(np.asarray(inputs["W_hh"], f32).astype(GEMM_NP))
    w_out = np.ascontiguousarray(np.asarray(inputs["W_out"], f32).astype(GEMM_NP))
    b_in = np.ascontiguousarray(np.asarray(inputs["b_in"], f32).reshape(E, 1))
    b_t = np.ascontiguousarray(np.asarray(inputs["b_t"], f32).reshape(E, 1))
    b_tv = np.ascontiguousarray(np.asarray(inputs["b_tv"], f32).reshape(E, 1))
    # per-chunk bias columns: [128, 8], chunk c = gate cols [128c, 128c+128)
    b_sum = np.ascontiguousarray(
        (np.asarray(inputs["b_ih"], f32) + np.asarray(inputs["b_hh"], f32))
        .reshape(8, 128).T)
    b_out = np.ascontiguousarray(np.asarray(inputs["b_out"], f32).reshape(O, 1))
    ident = np.eye(128, dtype=f32)

    shared = dict(w_in=w_in, w_t=w_t, w_tv=w_tv, w_ih=w_ih, w_hh=w_hh,
                  w_out=w_out, b_in_col=b_in, b_t_col=b_t, b_tv_col=b_tv,
                  b_sum_cols=b_sum, b_out_col=b_out, ident=ident)

    in_maps = []
    for d in range(NCORES):
        i0 = d * NL
        gp = gttc[:, i0:i0 + NL]                       # [T, 64, 512, 24]
        gp = np.ascontiguousarray(
            gp.reshape(T, NL, 2, N // 2, S).transpose(0, 2, 1, 4, 3)
            .reshape(T, 128, PFREE).astype(GRID_NP))
        gv = gttcv[:, i0:i0 + NL]                      # [T, 64, 64, 24]
        gv = np.ascontiguousarray(
            gv.reshape(T, NL, 2, V // 2, S).transpose(0, 2, 1, 4, 3)
            .reshape(T, 128, VFREE).astype(GRID_NP))
        nd = inp[:, i0:i0 + NL][:, :, [0, 1, 5, 6, 7, 8]]  # [T, 64, 6]
        nodes_T = np.ascontiguousarray(nd.reshape(ROWS, 6).T.astype(GEMM_NP))
        in_maps.append(dict(
            g_ped=gp, g_veh=gv, nodes_T=nodes_T,
            hT_init=np.ascontiguousarray(h0[i0:i0 + NL].T.astype(GEMM_NP)),
            cT_init=np.ascontiguousarray(c0[i0:i0 + NL].T),
            **shared))
    return in_maps


def gather_outputs(results):
    outs, hs, cs = [], [], []
    for r in results:
        # outs_T [5, 19*64] -> [19, 64, 5]
        o = r["outs_T"].reshape(O, T, NL).transpose(1, 2, 0)
        outs.append(o)
        hs.append(np.ascontiguousarray(r["hT_out"].T.astype(np.float32)))
        cs.append(np.ascontiguousarray(r["cT_out"].T))
    return (np.concatenate(outs, axis=1),
            np.concatenate(hs, axis=0),
            np.concatenate(cs, axis=0))


def kernel(**inputs):
    from concourse.bass_utils import run_bass_kernel_spmd
    nc = build_nc()
    in_maps = shard_inputs(inputs)
    res = run_bass_kernel_spmd(nc, in_maps, core_ids=list(range(NCORES)))
    return gather_outputs(res.results)


# revision 33
# speedup vs baseline: 1.1306x; 1.0882x over previous
"""Trainium2 Bass kernel for a collision-grid social-LSTM model.

Math per frame t (N=512 agents, V=64 vehicles):
  social   = max_j grids_TTC[t, :, j, :]          # [N, 24]
  social_v = max_j grids_TTC_veh[t, :, j, :]      # [N, 24]
  e_in = relu(nodes @ W_in + b_in)                # nodes = input_data[:, [0,1,5..8]]
  e_t  = relu(social @ W_t + b_t)
  e_tv = relu(social_v @ W_tv + b_tv)
  gates = [e_in e_t e_tv] @ W_ih + b_ih + h @ W_hh + b_hh
  LSTM cell (i,f,g,o) -> h, c;  out = h @ W_out + b_out

Sharding: agent dim N split across 8 NeuronCores (64 rows each); weights
replicated; the T-scan stays sequential per core; no collectives.

Grid streaming: each frame slab [64i, 512j, 24s] is reshaped on host to
[128, 6144] fp16 with partition p = (j_half*64 + i) and free layout
s-major [24, 256j], so the DMA is one contiguous transfer and the
j-reduction runs as a halving tensor-max tree at the DVE's 2x fp16 rate.
A PE transpose + elementwise max merges the two j-halves and produces
social^T [24, 64] directly.

Everything downstream runs TRANSPOSED (feature dims on partitions, agent
rows on the free axis): gates^T chunks [128 gate-dims, 64 rows] accumulate
in PSUM from stationary weight-chunk matmuls; biases ride the sigmoid/tanh
activations as free per-partition bias vectors; h^T is written straight
into an SBUF history buffer (no per-step transposes or copies); matmul
operands are fp16 while PSUM accumulation and the LSTM cell state stay
fp32.
"""

import numpy as np

import concourse.tile as tile
from concourse import bacc, mybir

T, N, V = 19, 512, 64
F, E, R, O = 9, 128, 256, 5
S = 24
NCORES = 8
NL = N // NCORES          # 64 agent rows per core
ROWS = T * NL             # 1216 (t-major row index = t*NL + i)
PFREE = (N // 2) * S      # 6144 free elems per partition (ped)
VFREE = (V // 2) * S      # 768 (veh)

DT = mybir.dt.float32
GRID_DT = mybir.dt.float16   # dtype grids are staged in device DRAM
GRID_NP = np.float16
GEMM_DT = mybir.dt.float16   # matmul operand dtype (PSUM accumulates fp32)
GEMM_NP = np.float16

_NC_CACHE = {}


def build_nc(repeat=1, parts="all"):
    """Build + compile the per-core Bass module (identical on all cores).

    parts: "all" | "grids" (stream+reduce only) | "scan" (no grid streaming)
    — reduced variants are for cost-model experiments only.
    """
    key = (repeat, parts)
    if key in _NC_CACHE:
        return _NC_CACHE[key]

    nc = bacc.Bacc("TRN2", target_bir_lowering=False, debug=False,
                   num_devices=NCORES)
    dt = DT
    AF = mybir.ActivationFunctionType
    ALU = mybir.AluOpType
    AX = mybir.AxisListType

    # ---- DRAM I/O ----
    g_ped = nc.dram_tensor("g_ped", [T, 128, PFREE], GRID_DT, kind="ExternalInput")
    g_veh = nc.dram_tensor("g_veh", [T, 128, VFREE], GRID_DT, kind="ExternalInput")
    nodes_T = nc.dram_tensor("nodes_T", [6, ROWS], GEMM_DT, kind="ExternalInput")
    hT_init = nc.dram_tensor("hT_init", [R, NL], GEMM_DT, kind="ExternalInput")
    cT_init = nc.dram_tensor("cT_init", [R, NL], dt, kind="ExternalInput")
    w_in_d = nc.dram_tensor("w_in", [6, E], GEMM_DT, kind="ExternalInput")
    w_t_d = nc.dram_tensor("w_t", [S, E], GEMM_DT, kind="ExternalInput")
    w_tv_d = nc.dram_tensor("w_tv", [S, E], GEMM_DT, kind="ExternalInput")
    b_in_d = nc.dram_tensor("b_in_col", [E, 1], dt, kind="ExternalInput")
    b_t_d = nc.dram_tensor("b_t_col", [E, 1], dt, kind="ExternalInput")
    b_tv_d = nc.dram_tensor("b_tv_col", [E, 1], dt, kind="ExternalInput")
    w_ih_d = nc.dram_tensor("w_ih", [3 * E, 4 * R], GEMM_DT, kind="ExternalInput")
    w_hh_d = nc.dram_tensor("w_hh", [R, 4 * R], GEMM_DT, kind="ExternalInput")
    # b_ih + b_hh regrouped host-side as [128, 8] (chunk-major columns)
    b_sum_d = nc.dram_tensor("b_sum_cols", [E, 8], dt, kind="ExternalInput")
    w_out_d = nc.dram_tensor("w_out", [R, O], GEMM_DT, kind="ExternalInput")
    b_out_d = nc.dram_tensor("b_out_col", [O, 1], dt, kind="ExternalInput")
    ident_d = nc.dram_tensor("ident", [128, 128], dt, kind="ExternalInput")

    outs_d = nc.dram_tensor("outs_T", [O, ROWS], dt, kind="ExternalOutput")
    h_d = nc.dram_tensor("hT_out", [R, NL], GEMM_DT, kind="ExternalOutput")
    c_d = nc.dram_tensor("cT_out", [R, NL], dt, kind="ExternalOutput")

    with tile.TileContext(nc) as tc:
        import contextlib
        with contextlib.ExitStack() as ctx:
            consts = ctx.enter_context(tc.tile_pool(name="consts", bufs=1))
            grids = ctx.enter_context(tc.tile_pool(name="grids", bufs=4))
            gridsv = ctx.enter_context(tc.tile_pool(name="gridsv", bufs=4))
            tmp = ctx.enter_context(tc.tile_pool(name="tmp", bufs=3))
            lstm = ctx.enter_context(tc.tile_pool(name="lstm", bufs=2))
            ps_gates = ctx.enter_context(
                tc.tile_pool(name="ps_gates", bufs=2, space="PSUM"))
            ps_small = ctx.enter_context(
                tc.tile_pool(name="ps_small", bufs=4, space="PSUM"))

            # ---- load constants ----
            def cload(name, dram, shape, dtype=dt):
                t_ = consts.tile(shape, dtype, tag=name)
                nc.sync.dma_start(t_[:], dram[:])
                return t_

            w_in = cload("w_in", w_in_d, [6, E], GEMM_DT)
            w_t = cload("w_t", w_t_d, [S, E], GEMM_DT)
            w_tv = cload("w_tv", w_tv_d, [S, E], GEMM_DT)
            b_in = cload("b_in", b_in_d, [E, 1])
            b_t = cload("b_t", b_t_d, [E, 1])
            b_tv = cload("b_tv", b_tv_d, [E, 1])
            # W_ih [384, 1024] as three [128, 1024] k-slabs
            w_ih0 = consts.tile([E, 4 * R], GEMM_DT, tag="w_ih0")
            nc.sync.dma_start(w_ih0[:], w_ih_d[0:E, :])
            w_ih1 = consts.tile([E, 4 * R], GEMM_DT, tag="w_ih1")
            nc.sync.dma_start(w_ih1[:], w_ih_d[E:2 * E, :])
            w_ih2 = consts.tile([E, 4 * R], GEMM_DT, tag="w_ih2")
            nc.sync.dma_start(w_ih2[:], w_ih_d[2 * E:3 * E, :])
            w_hh0 = consts.tile([128, 4 * R], GEMM_DT, tag="w_hh0")
            nc.sync.dma_start(w_hh0[:], w_hh_d[0:128, :])
            w_hh1 = consts.tile([128, 4 * R], GEMM_DT, tag="w_hh1")
            nc.sync.dma_start(w_hh1[:], w_hh_d[128:256, :])
            b_sum = cload("b_sum", b_sum_d, [E, 8])
            w_out0 = consts.tile([128, O], GEMM_DT, tag="w_out0")
            nc.sync.dma_start(w_out0[:], w_out_d[0:128, :])
            w_out1 = consts.tile([128, O], GEMM_DT, tag="w_out1")
            nc.sync.dma_start(w_out1[:], w_out_d[128:256, :])
            b_out = cload("b_out", b_out_d, [O, 1])
            ident = cload("ident", ident_d, [128, 128])
            nodesT = cload("nodesT", nodes_T, [6, ROWS], GEMM_DT)

            # persistent activations (transposed, rows on free dim)
            eT_in = consts.tile([E, ROWS], GEMM_DT, tag="eT_in")
            eT_t = consts.tile([E, ROWS], GEMM_DT, tag="eT_t")
            eT_tv = consts.tile([E, ROWS], GEMM_DT, tag="eT_tv")
            socT = consts.tile([S, ROWS], GEMM_DT, tag="socT")
            socvT = consts.tile([S, ROWS], GEMM_DT, tag="socvT")
            outsT_sb = consts.tile([O, ROWS], dt, tag="outsT_sb")
            # h^T history, both r-halves side by side per step block:
            # block t = cols [2t*NL, (2t+2)*NL): [h_half0 | h_half1]
            hist = consts.tile([128, 2 * (T + 1) * NL], GEMM_DT, tag="hist")
            if parts.startswith("scan"):
                nc.vector.memset(socT[:], 0.5)
                nc.vector.memset(socvT[:], 0.5)

            for _rep in range(repeat):
                # ---- e_in for all rows upfront ----
                for n0 in range(0, ROWS, 512):
                    nn = min(512, ROWS - n0)
                    ps = ps_small.tile([128, 512], dt, tag="ps")
                    nc.tensor.matmul(ps[:, :nn], w_in[:], nodesT[:, n0:n0 + nn],
                                     start=True, stop=True)
                    nc.scalar.activation(eT_in[:, n0:n0 + nn], ps[:, :nn],
                                         AF.Relu, bias=b_in[:], scale=1.0)

                # ---- LSTM state init (both r-halves side by side) ----
                nc.sync.dma_start(hist[:, 0:NL], hT_init[0:128, :])
                nc.sync.dma_start(hist[:, NL:2 * NL], hT_init[128:256, :])
                c_cur = lstm.tile([128, 2 * NL], dt, tag="c")
                nc.sync.dma_start(c_cur[:, 0:NL], cT_init[0:128, :])
                nc.sync.dma_start(c_cur[:, NL:2 * NL], cT_init[128:256, :])

                def emit_grids(t, cols):
                    # stream + reduce ped grid.  Free layout per partition
                    # is s-major [24, 256j]; halving tensor-max tree at the
                    # DVE's 2x fp16 rate, then one strided reduce finishes
                    # j=16 -> 1.
                    G = grids.tile([128, PFREE], GRID_DT, tag="G")
                    nc.sync.dma_start(G[:], g_ped[t])
                    scr = grids.tile([128, 5760], GRID_DT, tag="SCR")
                    lvls = [(G, 0, 256, scr, 0),          # -> [24,128] @0
                            (scr, 0, 128, scr, 3072),     # -> [24,64]  @3072
                            (scr, 3072, 64, scr, 4608),   # -> [24,32]  @4608
                            (scr, 4608, 32, scr, 5376)]   # -> [24,16]  @5376
                    for src_t, soff, jn, dst_t, doff in lvls:
                        jh = jn // 2
                        a = src_t[:, soff:soff + S * jn].rearrange(
                            "p (s j) -> p s j", s=S)
                        o = dst_t[:, doff:doff + S * jh].rearrange(
                            "p (s j) -> p s j", s=S)
                        nc.vector.tensor_max(o, a[:, :, 0:jh], a[:, :, jh:jn])
                    part = tmp.tile([128, S], dt, tag="part")
                    nc.vector.tensor_reduce(
                        part[:], scr[:, 5376:5760].rearrange(
                            "p (s j) -> p s j", s=S),
                        axis=AX.X, op=ALU.max)
                    psT = ps_small.tile([S, 128], dt, tag="ps")
                    nc.tensor.transpose(psT[:], part[:], ident[:])
                    sbT = tmp.tile([S, 128], dt, tag="sbT")
                    nc.scalar.copy(sbT[:], psT[:])
                    nc.vector.tensor_max(socT[:, cols], sbT[:, 0:NL],
                                         sbT[:, NL:2 * NL])
                    # stream + reduce veh grid (one tree level + reduce)
                    Gv = gridsv.tile([128, VFREE], GRID_DT, tag="Gv")
                    nc.sync.dma_start(Gv[:], g_veh[t])
                    scrv = gridsv.tile([128, VFREE // 2], GRID_DT, tag="SCRV")
                    av = Gv[:].rearrange("p (s j) -> p s j", s=S)
                    ov = scrv[:].rearrange("p (s j) -> p s j", s=S)
                    nc.vector.tensor_max(ov, av[:, :, 0:16], av[:, :, 16:32])
                    partv = tmp.tile([128, S], dt, tag="partv")
                    nc.vector.tensor_reduce(
                        partv[:], scrv[:].rearrange("p (s j) -> p s j", s=S),
                        axis=AX.X, op=ALU.max)
                    psTv = ps_small.tile([S, 128], dt, tag="ps")
                    nc.tensor.transpose(psTv[:], partv[:], ident[:])
                    sbTv = tmp.tile([S, 128], dt, tag="sbTv")
                    nc.scalar.copy(sbTv[:], psTv[:])
                    nc.vector.tensor_max(socvT[:, cols], sbTv[:, 0:NL],
                                         sbTv[:, NL:2 * NL])
                    # social embeddings (transposed)
                    pse = ps_small.tile([E, NL], dt, tag="ps")
                    nc.tensor.matmul(pse[:], w_t[:], socT[:, cols],
                                     start=True, stop=True)
                    nc.scalar.activation(eT_t[:, cols], pse[:], AF.Relu,
                                         bias=b_t[:], scale=1.0)
                    psev = ps_small.tile([E, NL], dt, tag="ps")
                    nc.tensor.matmul(psev[:], w_tv[:], socvT[:, cols],
                                     start=True, stop=True)
                    nc.scalar.activation(eT_tv[:, cols], psev[:], AF.Relu,
                                         bias=b_tv[:], scale=1.0)

                def emit_xw(t):
                    # input-side GEMM for step t: one accumulation group per
                    # PSUM bank (single start on the bank's first matmul; the
                    # h-side matmuls of step t will extend and stop it).
                    # Emitted during step t-1's ACT/DVE chain so the PE
                    # never idles at the h dependency.
                    cols = slice(t * NL, (t + 1) * NL)
                    gA = ps_gates.tile([128, 4 * NL], dt, tag="gA")
                    gB = ps_gates.tile([128, 4 * NL], dt, tag="gB")
                    for c in range(8):
                        g_ps = gA if c < 4 else gB
                        out = g_ps[:, (c % 4) * NL:(c % 4) * NL + NL]
                        wsl = slice(c * 128, (c + 1) * 128)
                        nc.tensor.matmul(out, w_ih0[:, wsl], eT_in[:, cols],
                                         start=(c % 4 == 0), stop=False)
                        nc.tensor.matmul(out, w_ih1[:, wsl], eT_t[:, cols],
                                         start=False, stop=False)
                        nc.tensor.matmul(out, w_ih2[:, wsl], eT_tv[:, cols],
                                         start=False, stop=False)
                    return gA, gB

                def gcols(t):
                    return slice(t * NL, (t + 1) * NL)

                LA = 2   # grid pipeline lookahead (frames) vs the scan
                if parts == "all":
                    for t in range(min(LA, T)):
                        emit_grids(t, gcols(t))
                    gA, gB = emit_xw(0)

                for t in range(T):
                    cols = slice(t * NL, (t + 1) * NL)
                    h0c = slice(2 * t * NL, (2 * t + 1) * NL)       # h_{t-1} r0
                    h1c = slice((2 * t + 1) * NL, (2 * t + 2) * NL)  # h_{t-1} r1
                    hnb = slice(2 * (t + 1) * NL, 2 * (t + 2) * NL)  # h_t block

                    if parts == "grids":
                        emit_grids(t, cols)
                        continue
                    if parts == "all" and t + LA < T:
                        emit_grids(t + LA, gcols(t + LA))

                    # ---- h-side GEMM extends the per-bank groups opened
                    # by emit_xw(t); stop on each bank's last matmul ----
                    act = tmp.tile([128, 8 * NL], dt, tag="act")
                    for c in range(8):
                        g_ps = gA if c < 4 else gB
                        out = g_ps[:, (c % 4) * NL:(c % 4) * NL + NL]
                        wsl = slice(c * 128, (c + 1) * 128)
                        nc.tensor.matmul(out, w_hh0[:, wsl], hist[:, h0c],
                                         start=False, stop=False)
                        nc.tensor.matmul(out, w_hh1[:, wsl], hist[:, h1c],
                                         start=False, stop=(c % 4 == 3))
                        if c % 4 == 3:
                            # bank group closed: activations for its 4 chunks
                            for cc in range(c - 3, c + 1):
                                o2 = g_ps[:, (cc % 4) * NL:(cc % 4) * NL + NL]
                                func = (AF.Tanh if cc in (4, 5)
                                        else AF.Sigmoid)
                                nc.scalar.activation(
                                    act[:, cc * NL:(cc + 1) * NL], o2,
                                    func, bias=b_sum[:, cc:cc + 1], scale=1.0)

                    # act cols (chunk-major): i [0:128], f [128:256],
                    # g [256:384], o [384:512] — each [h0 | h1], aligned
                    # with the [h0 | h1] layout of c_cur and hist blocks.
                    t1_ = tmp.tile([128, 2 * NL], dt, tag="t1")
                    nc.vector.tensor_mul(t1_[:], act[:, 128:256], c_cur[:])
                    t2_ = tmp.tile([128, 2 * NL], dt, tag="t2")
                    nc.vector.tensor_mul(t2_[:], act[:, 0:128],
                                         act[:, 256:384])
                    c_new = lstm.tile([128, 2 * NL], dt, tag="c")
                    nc.vector.tensor_add(c_new[:], t1_[:], t2_[:])
                    tanc = tmp.tile([128, 2 * NL], dt, tag="tanc")
                    nc.scalar.activation(tanc[:], c_new[:], AF.Tanh)
                    nc.vector.tensor_mul(hist[:, hnb], act[:, 384:512],
                                         tanc[:])
                    c_cur = c_new

                    # ---- out_t^T = W_out^T h_t + b_out ----
                    hn0 = slice(2 * (t + 1) * NL, (2 * t + 3) * NL)
                    hn1 = slice((2 * t + 3) * NL, 2 * (t + 2) * NL)
                    pso = ps_small.tile([O, NL], dt, tag="ps")
                    nc.tensor.matmul(pso[:], w_out0[:], hist[:, hn0],
                                     start=True, stop=False)
                    nc.tensor.matmul(pso[:], w_out1[:], hist[:, hn1],
                                     start=False, stop=True)
                    nc.scalar.activation(outsT_sb[:, cols], pso[:],
                                         AF.Identity, bias=b_out[:], scale=1.0)

                    if t + 1 < T:
                        gA, gB = emit_xw(t + 1)

            # ---- writeback ----
            if parts == "grids":
                nc.sync.dma_start(outs_d[0:O, 0:NL], socT[0:O, 0:NL])
            else:
                nc.sync.dma_start(outs_d[:], outsT_sb[:])
                nc.sync.dma_start(h_d[0:128, :],
                                  hist[:, 2 * T * NL:(2 * T + 1) * NL])
                nc.sync.dma_start(h_d[128:256, :],
                                  hist[:, (2 * T + 1) * NL:2 * (T + 1) * NL])
                nc.sync.dma_start(c_d[0:128, :], c_cur[:, 0:NL])
                nc.sync.dma_start(c_d[128:256, :], c_cur[:, NL:2 * NL])

    nc.compile()
    _NC_CACHE[key] = nc
    return nc


def shard_inputs(inputs):
    """Full inputs -> list of 8 per-core input maps (numpy, C-contiguous)."""
    f32 = np.float32
    inp = np.asarray(inputs["input_data"], f32)
    gttc = np.asarray(inputs["grids_TTC"], f32)
    gttcv = np.asarray(inputs["grids_TTC_veh"], f32)
    h0 = np.asarray(inputs["hidden_states"], f32)
    c0 = np.asarray(inputs["cell_states"], f32)

    w_in = np.ascontiguousarray(np.asarray(inputs["W_in"], f32).astype(GEMM_NP))
    w_t = np.ascontiguousarray(np.asarray(inputs["W_t"], f32).astype(GEMM_NP))
    w_tv = np.ascontiguousarray(np.asarray(inputs["W_tv"], f32).astype(GEMM_NP))
    w_ih = np.ascontiguousarray(np.asarray(inputs["W_ih"], f32).astype(GEMM_NP))
    w_hh = np.ascontiguousarray(np.asarray(inputs["W_hh"], f32).astype(GEMM_NP))
    w_out = np.ascontiguousarray(np.asarray(inputs["W_out"], f32).astype(GEMM_NP))
    b_in = np.ascontiguousarray(np.asarray(inputs["b_in"], f32).reshape(E, 1))
    b_t = np.ascontiguousarray(np.asarray(inputs["b_t"], f32).reshape(E, 1))
    b_tv = np.ascontiguousarray(np.asarray(inputs["b_tv"], f32).reshape(E, 1))
    # per-chunk bias columns: [128, 8], chunk c = gate cols [128c, 128c+128)
    b_sum = np.ascontiguousarray(
        (np.asarray(inputs["b_ih"], f32) + np.asarray(inputs["b_hh"], f32))
        .reshape(8, 128).T)
    b_out = np.ascontiguousarray(np.asarray(inputs["b_out"], f32).reshape(O, 1))
    ident = np.eye(128, dtype=f32)

    shared = dict(w_in=w_in, w_t=w_t, w_tv=w_tv, w_ih=w_ih, w_hh=w_hh,
                  w_out=w_out, b_in_col=b_in, b_t_col=b_t, b_tv_col=b_tv,
                  b_sum_cols=b_sum, b_out_col=b_out, ident=ident)

    in_maps = []
    for d in range(NCORES):
        i0 = d * NL
        gp = gttc[:, i0:i0 + NL]                       # [T, 64, 512, 24]
        gp = np.ascontiguousarray(
            gp.reshape(T, NL, 2, N // 2, S).transpose(0, 2, 1, 4, 3)
            .reshape(T, 128, PFREE).astype(GRID_NP))
        gv = gttcv[:, i0:i0 + NL]                      # [T, 64, 64, 24]
        gv = np.ascontiguousarray(
            gv.reshape(T, NL, 2, V // 2, S).transpose(0, 2, 1, 4, 3)
            .reshape(T, 128, VFREE).astype(GRID_NP))
        nd = inp[:, i0:i0 + NL][:, :, [0, 1, 5, 6, 7, 8]]  # [T, 64, 6]
        nodes_T = np.ascontiguousarray(nd.reshape(ROWS, 6).T.astype(GEMM_NP))
        in_maps.append(dict(
            g_ped=gp, g_veh=gv, nodes_T=nodes_T,
            hT_init=np.ascontiguousarray(h0[i0:i0 + NL].T.astype(GEMM_NP)),
            cT_init=np.ascontiguousarray(c0[i0:i0 + NL].T),
            **shared))
    return in_maps


def gather_outputs(results):
    outs, hs, cs = [], [], []
    for r in results:
        # outs_T [5, 19*64] -> [19, 64, 5]
        o = r["outs_T"].reshape(O, T, NL).transpose(1, 2, 0)
        outs.append(o)
        hs.append(np.ascontiguousarray(r["hT_out"].T.astype(np.float32)))
        cs.append(np.ascontiguousarray(r["cT_out"].T))
    return (np.concatenate(outs, axis=1),
            np.concatenate(hs, axis=0),
            np.concatenate(cs, axis=0))


def kernel(**inputs):
    from concourse.bass_utils import run_bass_kernel_spmd
    nc = build_nc()
    in_maps = shard_inputs(inputs)
    res = run_bass_kernel_spmd(nc, in_maps, core_ids=list(range(NCORES)))
    return gather_outputs(res.results)
